# revision 1
# baseline (speedup 1.0000x reference)
"""Multi-head attention layer on 8 Trainium2 NeuronCores.

Sharding (zero-communication): core c -> (batch c//2, head-group c%2), i.e.
each core owns 8 of the 16 heads (512 of 1024 hidden dims) for one batch
element.  Per core: QKV projections for its heads, full softmax attention,
and a partial output projection (row-parallel over Wo).  The host sums the
two partial outputs per batch and adds the constant bias terms
(bo + bv @ Wo.T -- the value bias commutes through softmax since attention
rows sum to 1), so no on-device collectives are needed.

dtypes: all matmul operands fp16 (1 PE cycle/row, normal PE mode so the HAM
clock-gate stays at 2.4 GHz, 10-bit mantissa); PSUM accumulation, softmax
sums and normalization in fp32.  End-to-end max error ~2.6e-4 of |out|max.

Per-core device schedule (single NEFF, Tile-framework scheduled):
  qT,kT [512 dh, 2048 tok] head-major on partitions; v1 [s, 8 heads, 65]
  with a ones column so the AV matmul emits softmax sums for free.
  Attention runs per head-PAIR: even head's scores on PE array tile T0
  (partitions 0:64), odd head's on T8 (64:128) -- both 64-row matmuls
  execute concurrently on the row-tiled 128x128 array.
  scoresT [128 s, 1024 l] PSUM -> exp on ACT (the ~290us bottleneck) ->
  P fp16 -> AV accumulate [65, 1024] over 16 s-tiles.
  1/sum is computed with a fast reciprocal (input must sit at partition 0 --
  the custom DVE op misreads partition-64 inputs), broadcast across 64
  partitions via a DRAM round-trip DMA, applied at AV eviction.
  Emission order = Tile trace order: projections are emitted before the
  attention that reads them; out-proj(lc0) is spread between lc1 pairs.
  Measured ~483 us on HW (profiled core 0), max rel err 2.6e-4.
"""

import os
import numpy as np

B, L, S = 4, 2048, 2048
D, NH, E = 1024, 16, 64
N_CORES = 8
HG = 2
LH = NH // HG         # 8 local heads
DH = LH * E           # 512
LC = 1024
NLC = L // LC
SCALE = 1.0 / np.sqrt(E)

_compiled = {}
last_exec_time_ns = None
last_results = None


def _round_fp32r(x: np.ndarray) -> np.ndarray:
    x = np.ascontiguousarray(x, dtype=np.float32)
    u = x.view(np.uint32)
    keep = np.uint32(0xFFFFF800)
    half = np.uint32(0x400)
    lsb = (u >> np.uint32(11)) & np.uint32(1)
    r = (u + half - np.uint32(1) + lsb) & keep
    return r.view(np.float32)


def _build():
    import concourse.bass as bass
    import concourse.mybir as mybir
    import concourse.tile as tile
    from concourse import bacc

    f32 = mybir.dt.float32
    f32r = mybir.dt.float32r
    bf16 = mybir.dt.float16

    nc = bacc.Bacc("TRN2", target_bir_lowering=False, debug=False,
                   num_devices=N_CORES)

    xqT = nc.dram_tensor("xqT", [D, L], bf16, kind="ExternalInput").ap()
    xkT = nc.dram_tensor("xkT", [D, S], bf16, kind="ExternalInput").ap()
    xvT = nc.dram_tensor("xvT", [D, S], bf16, kind="ExternalInput").ap()
    wqT = nc.dram_tensor("wqT", [D, DH], bf16, kind="ExternalInput").ap()
    wkT = nc.dram_tensor("wkT", [D, DH], bf16, kind="ExternalInput").ap()
    wvT = nc.dram_tensor("wvT", [D, DH], bf16, kind="ExternalInput").ap()
    woT = nc.dram_tensor("woT", [DH, D], bf16, kind="ExternalInput").ap()
    bq_d = nc.dram_tensor("bq", [DH], f32, kind="ExternalInput").ap()
    bk_d = nc.dram_tensor("bk", [DH], f32, kind="ExternalInput").ap()
    out_d = nc.dram_tensor("out", [L, D], f32, kind="ExternalOutput").ap()

    Exp = mybir.ActivationFunctionType.Exp

    with tile.TileContext(nc) as tc:
        with (
            tc.tile_pool(name="res", bufs=1) as res,
            tc.tile_pool(name="xs", bufs=10) as xs,
            tc.tile_pool(name="ws", bufs=6) as ws,
            tc.tile_pool(name="pp", bufs=8) as pp,
            tc.tile_pool(name="os", bufs=4) as osp,
            tc.tile_pool(name="sm", bufs=3) as sm,
            tc.tile_pool(name="sm2", bufs=3) as sm2,
            tc.tile_pool(name="dr", bufs=3, space="DRAM") as dr,
            tc.tile_pool(name="psA", bufs=2, space="PSUM") as psA,
            tc.tile_pool(name="psB", bufs=2, space="PSUM") as psB,
        ):
            # ---- constants / resident weights ----
            bq_sb = res.tile([128, DH // 128], f32, tag="bq")
            bk_sb = res.tile([128, DH // 128], f32, tag="bk")
            nc.sync.dma_start(bq_sb[:], bq_d.rearrange("(c p) -> p c", p=128))
            nc.sync.dma_start(bk_sb[:], bk_d.rearrange("(c p) -> p c", p=128))
            wo_sb = res.tile([128, DH // 128, D], bf16, tag="wo")
            nc.sync.dma_start(wo_sb[:], woT.rearrange("(c p) n -> p c n", p=128))
            wv_sb = res.tile([128, D // 128, DH], bf16, tag="wv")
            nc.sync.dma_start(wv_sb[:], wvT.rearrange("(c p) n -> p c n", p=128))
            wq_r = res.tile([128, D // 128, DH], bf16, tag="wqr")
            nc.sync.dma_start(wq_r[:], wqT.rearrange("(c p) n -> p c n", p=128))
            wk_r = res.tile([128, D // 128, DH], bf16, tag="wkr")
            nc.sync.dma_start(wk_r[:], wkT.rearrange("(c p) n -> p c n", p=128))
            ones_f = res.tile([128, 128], f32, tag="onesf")
            nc.vector.memset(ones_f[:], 1.0)

            qT_sb = res.tile([128, DH // 128, L], bf16, tag="qT")
            kT_sb = res.tile([128, DH // 128, S], bf16, tag="kT")
            v1_sb = res.tile([128, S // 128, LH, E + 1], bf16, tag="v1")
            nc.vector.tensor_copy(
                v1_sb[:, :, :, E:E + 1],
                ones_f[:, 0:S // 128 * LH].rearrange(
                    "p (s h o) -> p s h o", h=LH, o=1))

            def v_proj_block(sb4):
                xv_t = []
                for d in range(8):
                    t = xs.tile([128, 512], bf16, tag="xs")
                    nc.sync.dma_start(
                        t[:], xvT[d * 128:(d + 1) * 128,
                                  sb4 * 512:(sb4 + 1) * 512])
                    xv_t.append(t)
                for st4 in range(4):
                    st = sb4 * 4 + st4
                    vp = psB.tile([128, DH], f32, tag="B")
                    for d in range(8):
                        nc.tensor.matmul(
                            vp[:], xv_t[d][:, st4 * 128:(st4 + 1) * 128],
                            wv_sb[:, d, :], start=(d == 0), stop=(d == 7))
                    nc.vector.tensor_copy(
                        out=v1_sb[:, st, :, 0:E],
                        in_=vp.rearrange("p (h e) -> p h e", h=LH))

            def kq_proj_block(x_in, w_r, b_sb, dst, bl):
                x_t = []
                for d in range(8):
                    t = xs.tile([128, 512], bf16, tag="xsb")
                    nc.sync.dma_start(
                        t[:], x_in[d * 128:(d + 1) * 128,
                                   bl * 512:(bl + 1) * 512])
                    x_t.append(t)
                for dh in range(4):
                    prj = psA.tile([128, 512], f32, tag="A")
                    for d in range(8):
                        nc.tensor.matmul(
                            prj[:], w_r[:, d, dh * 128:(dh + 1) * 128],
                            x_t[d][:], start=(d == 0), stop=(d == 7))
                    nc.vector.tensor_scalar_add(
                        out=dst[:, dh, bl * 512:(bl + 1) * 512],
                        in0=prj[:], scalar1=b_sb[:, dh:dh + 1])

            attT = {}

            def _normalize(lc, h, av):
                dhc, po = h // 2, (h % 2) * 64
                av_sb = sm2.tile([E + 1, LC], f32, tag="av", name="av_sb")
                nc.vector.tensor_copy(av_sb[:], av[:])
                sums0 = sm.tile([1, LC], f32, tag="sums0", name="sums0")
                nc.vector.tensor_copy(sums0[:], av_sb[E:E + 1, :])
                rec = sm.tile([1, LC], f32, tag="rec", name="rec")
                scr = sm.tile([1, LC], f32, tag="scr", name="scr")
                nc.vector.reciprocal_approx_accurate(rec[:], sums0[:], scr[:])
                rec_d = dr.tile([LC], f32, tag="recd", name="rec_d")
                nc.sync.dma_start(
                    rec_d[:].rearrange("(o l) -> o l", o=1), rec[:])
                rb_sb = sm2.tile([64, LC], f32, tag="rb", name="rb_sb")
                bcast = bass.AP(tensor=rec_d.tensor, offset=rec_d.offset,
                                ap=[[0, 64]] + list(rec_d.ap))
                nc.sync.dma_start(rb_sb[:], bcast)
                nc.vector.tensor_mul(attT[lc][po:po + 64, dhc, :],
                                     av_sb[0:E, :], rb_sb[:])

            def attention_pair(lc, c):
                # heads 2c (PE tile T0, partitions 0:64) and 2c+1 (T8,
                # partitions 64:128) run concurrently on the row-tiled array
                h0, h1 = 2 * c, 2 * c + 1
                av0 = psB.tile([E + 1, LC], f32, tag="B", name="av0")
                av1 = psB.tile([E + 1, LC], f32, tag="B", name="av1")
                for st in range(S // 128):
                    sc0 = psA.tile([128, LC], f32, tag="A", name="sc0")
                    sc1 = psA.tile([128, LC], f32, tag="A", name="sc1")
                    for nh in range(LC // 512):
                        lo = lc * LC + nh * 512
                        nc.tensor.matmul(
                            sc0[:, nh * 512:(nh + 1) * 512],
                            kT_sb[0:64, c, st * 128:(st + 1) * 128],
                            qT_sb[0:64, c, lo:lo + 512],
                            start=True, stop=True)
                        nc.tensor.matmul(
                            sc1[:, nh * 512:(nh + 1) * 512],
                            kT_sb[64:128, c, st * 128:(st + 1) * 128],
                            qT_sb[64:128, c, lo:lo + 512],
                            start=True, stop=True)
                    P0 = pp.tile([128, LC], bf16, tag="P", name="P0")
                    nc.scalar.activation(P0[:], sc0[:], Exp, scale=SCALE)
                    P1 = pp.tile([128, LC], bf16, tag="P", name="P1")
                    nc.scalar.activation(P1[:], sc1[:], Exp, scale=SCALE)
                    last = (st == S // 128 - 1)
                    for nh in range(LC // 512):
                        nc.tensor.matmul(
                            av0[:, nh * 512:(nh + 1) * 512],
                            v1_sb[:, st, h0, :],
                            P0[:, nh * 512:(nh + 1) * 512],
                            start=(st == 0), stop=last)
                    for nh in range(LC // 512):
                        nc.tensor.matmul(
                            av1[:, nh * 512:(nh + 1) * 512],
                            v1_sb[:, st, h1, :],
                            P1[:, nh * 512:(nh + 1) * 512],
                            start=(st == 0), stop=last)
                _normalize(lc, h0, av0)
                _normalize(lc, h1, av1)

            def out_proj(lc, ls_range=None):
                for ls in (ls_range if ls_range is not None
                           else range(LC // 128)):
                    for n2 in range(D // 512):
                        op = psB.tile([128, 512], f32, tag="B")
                        for dhc in range(DH // 128):
                            nc.tensor.matmul(
                                op[:],
                                attT[lc][:, dhc, ls * 128:(ls + 1) * 128],
                                wo_sb[:, dhc, n2 * 512:(n2 + 1) * 512],
                                start=(dhc == 0), stop=(dhc == DH // 128 - 1))
                        o_sb = osp.tile([128, 512], f32, tag="o")
                        nc.vector.tensor_copy(o_sb[:], op[:])
                        row = lc * LC + ls * 128
                        nc.sync.dma_start(
                            out_d[row:row + 128, n2 * 512:(n2 + 1) * 512],
                            o_sb[:])

            # ---- emission order: interleave projections with attention ----
            attT[0] = res.tile([128, DH // 128, LC], bf16, tag="attT0", name="attT0")
            attT[1] = res.tile([128, DH // 128, LC], bf16, tag="attT1", name="attT1")
            # all of V and K plus Q blocks 0-1 must be emitted (written)
            # before any lc0 attention reads them (trace-order dependencies)
            for bl in range(4):
                v_proj_block(bl)
                kq_proj_block(xkT, wk_r, bk_sb, kT_sb, bl)
            kq_proj_block(xqT, wq_r, bq_sb, qT_sb, 0)
            kq_proj_block(xqT, wq_r, bq_sb, qT_sb, 1)
            kq_proj_block(xqT, wq_r, bq_sb, qT_sb, 2)
            kq_proj_block(xqT, wq_r, bq_sb, qT_sb, 3)
            attention_pair(0, 0)
            attention_pair(0, 1)
            attention_pair(0, 2)
            attention_pair(0, 3)
            attention_pair(1, 0)
            out_proj(0, range(0, 2))
            attention_pair(1, 1)
            out_proj(0, range(2, 4))
            attention_pair(1, 2)
            out_proj(0, range(4, 6))
            attention_pair(1, 3)
            out_proj(0, range(6, 8))
            out_proj(1)

    nc.compile()
    return nc


def _get_nc():
    if "nc" not in _compiled:
        _compiled["nc"] = _build()
    return _compiled["nc"]


def kernel(queries, keys, values, Wq, bq, Wk, bk, Wv, bv, Wo, bo):
    global last_exec_time_ns, last_results
    from concourse import bass_utils

    queries = np.asarray(queries, dtype=np.float32)
    keys = np.asarray(keys, dtype=np.float32)
    values = np.asarray(values, dtype=np.float32)
    Wq, bq = np.asarray(Wq, np.float32), np.asarray(bq, np.float32)
    Wk, bk = np.asarray(Wk, np.float32), np.asarray(bk, np.float32)
    Wv, bv = np.asarray(Wv, np.float32), np.asarray(bv, np.float32)
    Wo, bo = np.asarray(Wo, np.float32), np.asarray(bo, np.float32)

    nc = _get_nc()

    in_maps = []
    for c in range(N_CORES):
        b, g = c // HG, c % HG
        sl = slice(g * DH, (g + 1) * DH)
        in_maps.append({
            "xqT": np.ascontiguousarray(queries[b].T).astype(np.float16),
            "xkT": np.ascontiguousarray(keys[b].T).astype(np.float16),
            "xvT": np.ascontiguousarray(values[b].T).astype(np.float16),
            "wqT": np.ascontiguousarray(Wq[sl, :].T).astype(np.float16),
            "wkT": np.ascontiguousarray(Wk[sl, :].T).astype(np.float16),
            "wvT": np.ascontiguousarray(Wv[sl, :].T).astype(np.float16),
            "woT": np.ascontiguousarray(Wo[:, sl].T).astype(np.float16),
            "bq": np.ascontiguousarray(bq[sl]),
            "bk": np.ascontiguousarray(bk[sl]),
        })

    trace = bool(os.environ.get("KERNEL_TRACE"))
    if trace:
        try:
            import antenv.axon_hooks  # noqa: F401
        except ImportError:
            trace = False
    res = bass_utils.run_bass_kernel_spmd(
        nc, in_maps, core_ids=list(range(N_CORES)), trace=trace)
    last_exec_time_ns = res.exec_time_ns
    last_results = res

    const = (bo + bv @ Wo.T).astype(np.float32)
    out = np.empty((B, L, D), np.float32)
    for b in range(B):
        out[b] = res.results[HG * b]["out"] + res.results[HG * b + 1]["out"] + const
    return out

